# revision 27
# baseline (speedup 1.0000x reference)
"""AugmentedConv3D Trainium2 kernel (v2: fp8 DoubleRow accum + split exp).

Reference computation (B=2, Cin=32, D=H=W=16, DK=32, DV=16, NH=4, KS=3):
  conv_out = conv3d(x, Wc, bc)            # (B, 48, 16,16,16)
  qkv      = conv3d(x, Wqkv, bqkv)        # (B, 80, 16,16,16)
  per head h: logits = (q_h/sqrt(8))^T k_h over P=4096 positions
              attn   = softmax(logits) @ v_h^T        # (P, 4)
  attn reshaped (faithful reshape, not transpose) to (B, 16, D,H,W),
  1x1x1 conv Wo/bo, concat with conv_out on channel axis.

Sharding: one core per (batch b, head h) pair = 8 cores.

Engine plan per core:
  PE:   fused conv (27 taps as 9 K=128 matmuls; q|conv0|k|conv1|v|ones x4),
        v-transposes, logits^T tiles [keys=128, queries] in f32r,
        softmax-weight accumulation in fp8 DoubleRow mode (2 key-tiles and
        2 weight bytes per partition-cycle), Wo matmuls.
  ACT:  conv-phase PSUM->SBUF copies, ~60% of the exp tiles
        (Exp with fused 1/sqrt(8) scale and -C bias, output e5m2),
        o8->attn8 staging copies.
  DVE:  f32->f32r input casts, q/k row extraction, v-transpose scaling to
        e4m3, ~40% of exp tiles via a Schraudolph bit-trick (affine to uint8,
        bitcast e5m2; max |weight| error ~8%, averaged out over the softmax),
        normalize (reciprocal + multiply).
  Pool: zero-fills of the padded q/k tiles.
  Weights w are exp((l - C)/sqrt(8)) in e5m2 (C=8.5 fixed shift; numerator
  and denominator both scale by e^-C so the softmax ratio is exact).
  v rows are scaled by 32 into e4m3; the 4 duplicated ones-channels give 4
  copies of the denominator row so the phase-C regroup lands on legal base
  partitions (0/32/64/96) without partition-strided engine reads.
Host: picks conv_out from one core per batch, sums the 4 head partials
(+bo) per batch and reassembles the (2, 64, 16, 16, 16) output.
"""
from contextlib import ExitStack

import numpy as np

import concourse.bacc as bacc
import concourse.tile as tile
from concourse import mybir
from concourse.bass_utils import run_bass_kernel_spmd

F32 = mybir.dt.float32
F32R = mybir.dt.float32r
U8 = mybir.dt.uint8
E3 = mybir.dt.float8e3
E4 = mybir.dt.float8e4
E5 = mybir.dt.float8e5

DK, DV, NH, KS = 32, 16, 4, 3
B, CIN, DIM = 2, 32, 16
P = DIM * DIM * DIM            # 4096
DKH, DVH = DK // NH, DV // NH  # 8, 4
NCO = 72                       # conv out channels: q|conv0|k|conv1|v|ones x4
PAD = DIM + 2                  # 18
SPAT = PAD * PAD * PAD         # 5832
WCC = 9 * NCO                  # 648
XS_OFF = WCC + 64 + 8          # 720 (block-diag Wo [16,64] + ident8)
XW_COLS = XS_OFF + SPAT        # 6552
S8 = float(DKH) ** -0.5

C_SHIFT = 8.5                  # exp shift: w = exp(l*S8 - C); cancels in ratio
VSCALE = 32.0                  # v and ones rows scaled into e4m3
SCH_A = (4.0 / np.log(2.0)) * S8          # Schraudolph scale on raw logits
SCH_B = 60.0 - 0.5 - (4.0 / np.log(2.0)) * C_SHIFT  # +60=4*15 bias, c=-0.5

_NC_CACHE = []


def _build_module(repeat=1):
    nc = bacc.Bacc("TRN2", target_bir_lowering=False, debug=False, num_devices=8)
    xw = nc.dram_tensor("xw", (128, XW_COLS), F32, kind="ExternalInput").ap()
    conv_out_d = nc.dram_tensor("conv_out", (48, P), F32, kind="ExternalOutput").ap()
    wo_part_d = nc.dram_tensor("wo_part", (4, 16, 1024), F32, kind="ExternalOutput").ap()

    with tile.TileContext(nc) as tc:
      for _rep in range(repeat):
        ctx = ExitStack()
        sbp = ctx.enter_context(tc.tile_pool(name="sb", bufs=1))

        # ---- input: 4 parallel DMAs, then f32 -> f32r casts (ACT + Pool) ----
        phin = ctx.enter_context(tc.tile_pool(name="phin", bufs=1))
        xwf = phin.tile([128, XW_COLS], F32)
        xwrt = sbp.tile([128, XW_COLS], F32R)
        qtp = sbp.tile([128, P], F32R)        # q rows 0:8, zeros below
        ktp = sbp.tile([128, P], F32R)        # k rows 0:8, zeros below
        CH = XW_COLS // 4
        for i in range(4):
            c0, c1 = i * CH, (i + 1) * CH
            nc.sync.dma_start(xwf[:, c0:c1], xw[:, c0:c1])
        nc.gpsimd.memset(qtp[:].bitcast(F32), 0.0)
        nc.gpsimd.memset(ktp[:].bitcast(F32), 0.0)
        for i in range(4):
            c0, c1 = i * CH, (i + 1) * CH
            if i < 1:
                nc.scalar.copy(xwrt[:, c0:c1], xwf[:, c0:c1])
            else:
                nc.gpsimd.tensor_copy(xwrt[:, c0:c1], xwf[:, c0:c1])
        bias_t = sbp.tile([128, 1], F32)
        nc.vector.tensor_scalar(bias_t[:], xwf[:, 0:1], 0.0, -C_SHIFT,
                                mybir.AluOpType.mult, mybir.AluOpType.add)

        xwr = xwrt[:]
        wconv9 = xwr[:, 0:WCC].rearrange("p (j co) -> p j co", j=9)
        wot4 = xwr[0:16, WCC:WCC + 64]        # block-diag, 4x [4,16]
        ident8_64 = xwr[64:72, WCC + 64:WCC + 72]
        xs4 = xwr[:, XS_OFF:XS_OFF + SPAT].rearrange(
            "p (a b c) -> p a b c", a=PAD, b=PAD, c=PAD)

        cstage = sbp.tile([NCO, P], F32R)     # q|conv0|k|conv1|v|ones
        vpt = sbp.tile([128, 32, 16], E4)     # [m%128, m//128, (v0..3,1,1,1,1)]*32
        nc.gpsimd.memset(vpt[:].bitcast(U8), 0)   # cols 8:16 zero (DR pad M=16)
        attn8 = sbp.tile([8, P], F32)         # rows 0-3 unnorm attn^T, 4-7 Z
        vgz = sbp.tile([16, 2048], F32)       # regroup: part 4r+j, cols v|z
        zrall = sbp.tile([16, 1024], F32)     # 1/Z per group
        drnall = sbp.tile([16, 1024], F32R)   # normalized attn groups

        # ---- fused conv + v' transpose + attention (one overlapped phase) ----
        with ExitStack() as phAB:
            cps = phAB.enter_context(tc.tile_pool(name="cps", bufs=1, space="PSUM"))
            vtp = phAB.enter_context(tc.tile_pool(name="vtp", bufs=1, space="PSUM"))
            lgp = phAB.enter_context(tc.tile_pool(name="lg", bufs=2, space="PSUM"))
            o8p = phAB.enter_context(tc.tile_pool(name="o8", bufs=1, space="PSUM"))
            wqp = phAB.enter_context(tc.tile_pool(name="wq", bufs=3))
            pvt = vtp.tile([128, 256], F32R)
            for t in range(8):
                cp = cps.tile([NCO, 512], F32)
                for j in range(9):
                    kd, kh = divmod(j, 3)
                    nc.tensor.matmul(
                        cp[:], wconv9[:, j, :],
                        xs4[:, 2 * t + kd:2 * t + kd + 2, kh:kh + DIM, 0:DIM],
                        start=(j == 0), stop=(j == 8))
                sl = np.s_[t * 512:(t + 1) * 512]
                nc.scalar.copy(cstage[:, sl], cp[:])
                for mm in range(4):
                    m = 4 * t + mm
                    nc.tensor.transpose(
                        pvt[:, 8 * m:8 * m + 8],
                        cstage[64:72, m * 128:(m + 1) * 128], ident8_64)
                nc.vector.tensor_scalar_mul(
                    vpt[:, 4 * t:4 * t + 4, 0:8],
                    pvt[:, 32 * t:32 * t + 32].rearrange("p (m c) -> p m c", c=8),
                    VSCALE)
                # q/k slices to padded base-0 tiles
                nc.vector.tensor_copy(qtp[0:8, sl], cp[0:8])
                nc.vector.tensor_copy(ktp[0:8, sl], cp[32:40])
            nc.sync.dma_start(conv_out_d[0:24], cstage[8:32, :].bitcast(F32))
            nc.sync.dma_start(conv_out_d[24:48], cstage[40:64, :].bitcast(F32))

            for qq in range(4):
                o16 = o8p.tile([16, 1024], F32)

                def accum(mp, wq):
                    for c in range(2):
                        nc.tensor.matmul(
                            o16[:, c * 512:(c + 1) * 512],
                            vpt[:, 2 * mp:2 * mp + 2, :],
                            wq[:, :, c * 512:(c + 1) * 512].bitcast(E5),
                            start=(mp == 0), stop=(mp == 15),
                            perf_mode=mybir.MatmulPerfMode.DoubleRow)

                prev = None       # delay accum one step so exp latency hides
                for mp in range(16):
                    wq = wqp.tile([128, 2, 1024], U8)
                    for i in range(2):
                        m = 2 * mp + i
                        lg = lgp.tile([128, 1024], F32)
                        k_ap = ktp[:, m * 128:(m + 1) * 128]
                        nc.tensor.matmul(lg[:, 0:512], k_ap,
                                         qtp[:, qq * 1024:qq * 1024 + 512],
                                         start=True, stop=True)
                        nc.tensor.matmul(lg[:, 512:1024], k_ap,
                                         qtp[:, qq * 1024 + 512:(qq + 1) * 1024],
                                         start=True, stop=True)
                        if m % 16 in (0, 2, 4, 6, 9, 11, 13):  # 14/32 on DVE
                            nc.vector.tensor_scalar(
                                wq[:, i, :], lg[:], SCH_A, SCH_B,
                                mybir.AluOpType.mult, mybir.AluOpType.add)
                        else:
                            nc.scalar.activation(
                                wq[:, i, :].bitcast(E5), lg[:],
                                mybir.ActivationFunctionType.Exp,
                                scale=S8, bias=bias_t[:])
                    if prev is not None:
                        accum(*prev)
                    prev = (mp, wq)
                accum(*prev)
                sl = np.s_[qq * 1024:(qq + 1) * 1024]
                nc.scalar.copy(attn8[:, sl], o16[0:8, :])
                # regroup DMAs: v rows -> partition 4r+qq cols 0:1024,
                #               z rows -> partition 4r+qq cols 1024:2048
                vview = vgz[:].rearrange("(r x) t -> r x t", r=4)
                nc.sync.dma_start(vview[:, qq, 0:1024], attn8[0:4, sl])
                nc.sync.dma_start(vview[:, qq, 1024:2048], attn8[4:8, sl])

        # ---- phase C: normalize + block-diag Wo, split by column halves ----
        # so recip/mul/matmul/copy/DMA pipeline across the two halves
        with ExitStack() as phC:
            pop = phC.enter_context(tc.tile_pool(name="po", bufs=1, space="PSUM"))
            wos = sbp.tile([64, 1024], F32)
            po = pop.tile([64, 1024], F32)
            wo_dst = wo_part_d[:].rearrange("r o t -> (r o) t")
            for hh in range(2):
                cs = np.s_[hh * 512:(hh + 1) * 512]
                nc.vector.reciprocal_approx_fast(
                    zrall[:, cs], vgz[:, 1024 + hh * 512:1536 + hh * 512])
                nc.vector.tensor_mul(drnall[:, cs], vgz[:, cs], zrall[:, cs])
                nc.tensor.matmul(po[:, cs], wot4, drnall[:, cs],
                                 start=True, stop=True)
                nc.scalar.copy(wos[:, cs], po[:, cs])
                nc.sync.dma_start(wo_dst[:, cs], wos[:, cs])
        ctx.close()

    nc.compile()
    return nc


def _build_null_module():
    """Tiny do-nothing module used by test.py to measure dispatch overhead."""
    nc = bacc.Bacc("TRN2", target_bir_lowering=False, debug=False, num_devices=8)
    nin = nc.dram_tensor("nin", (1, 16), F32, kind="ExternalInput").ap()
    nout = nc.dram_tensor("nout", (1, 16), F32, kind="ExternalOutput").ap()
    with tile.TileContext(nc) as tc, ExitStack() as ctx:
        p = ctx.enter_context(tc.tile_pool(name="p", bufs=1))
        t = p.tile([1, 16], F32)
        nc.sync.dma_start(t[:], nin[:])
        t2 = p.tile([1, 16], F32)
        nc.vector.tensor_copy(t2[:], t[:])
        nc.sync.dma_start(nout[:], t2[:])
    nc.compile()
    return nc


def _prep_core_input(x, Wc, bc, Wqkv, bqkv, Wo, b, h):
    """Build the [128, XW_COLS] f32 input blob for core (b, h)."""
    xpad = np.zeros((CIN, PAD, PAD, PAD), np.float32)
    xpad[:, 1:17, 1:17, 1:17] = x[b]
    flat = xpad.reshape(CIN, SPAT)
    xs = np.zeros((128, SPAT), np.float32)
    for kw in range(3):
        xs[kw * 32:(kw + 1) * 32, 0:SPAT - kw] = flat[:, kw:]
    xs[96] = 1.0

    Wsel = np.zeros((NCO, CIN, 3, 3, 3), np.float32)
    bsel = np.zeros((NCO,), np.float32)
    Wsel[0:8] = Wqkv[h * 8:(h + 1) * 8]
    bsel[0:8] = bqkv[h * 8:(h + 1) * 8]
    Wsel[8:32] = Wc[0:24]
    bsel[8:32] = bc[0:24]
    Wsel[32:40] = Wqkv[DK + h * 8:DK + (h + 1) * 8]
    bsel[32:40] = bqkv[DK + h * 8:DK + (h + 1) * 8]
    Wsel[40:64] = Wc[24:48]
    bsel[40:64] = bc[24:48]
    Wsel[64:68] = Wqkv[2 * DK + h * 4:2 * DK + (h + 1) * 4]
    bsel[64:68] = bqkv[2 * DK + h * 4:2 * DK + (h + 1) * 4]
    bsel[68:72] = 1.0                            # 4 ones channels -> 4 Z rows
    # [kw*32+ci, kd*3+kh, co]
    w9 = Wsel.transpose(4, 1, 2, 3, 0).reshape(96, 9, NCO)
    wconv = np.zeros((128, 9, NCO), np.float32)
    wconv[0:96] = w9
    wconv[96, 0, :] = bsel

    Wo16 = Wo[:, :, 0, 0, 0]                      # [16, 16]
    wot = np.ascontiguousarray(Wo16[:, 4 * h:4 * h + 4].T)  # [4, 16]

    xw = np.zeros((128, XW_COLS), np.float32)
    xw[:, 0:WCC] = wconv.reshape(128, WCC)
    for r in range(4):
        xw[4 * r:4 * r + 4, WCC + 16 * r:WCC + 16 * (r + 1)] = wot
    xw[64:72, WCC + 64:WCC + 72] = np.eye(8, dtype=np.float32)
    xw[:, XS_OFF:XS_OFF + SPAT] = xs
    return xw


def kernel(x, Wc, bc, Wqkv, bqkv, Wo, bo):
    x = np.asarray(x, np.float32)
    Wc = np.asarray(Wc, np.float32)
    bc = np.asarray(bc, np.float32)
    Wqkv = np.asarray(Wqkv, np.float32)
    bqkv = np.asarray(bqkv, np.float32)
    Wo = np.asarray(Wo, np.float32)
    bo = np.asarray(bo, np.float32)

    if not _NC_CACHE:
        _NC_CACHE.append(_build_module())
    nc = _NC_CACHE[0]

    in_maps = [
        {"xw": _prep_core_input(x, Wc, bc, Wqkv, bqkv, Wo, c // 4, c % 4)}
        for c in range(8)
    ]
    res = run_bass_kernel_spmd(nc, in_maps, core_ids=list(range(8)))

    out = np.empty((B, 64, DIM, DIM, DIM), np.float32)
    for b in range(B):
        out[b, 0:48] = res.results[4 * b]["conv_out"].reshape(48, DIM, DIM, DIM)
        acc = np.zeros((16, P), np.float32)
        for h in range(NH):
            wp = res.results[4 * b + h]["wo_part"]      # [4, 16, 1024]
            acc += wp.transpose(1, 2, 0).reshape(16, P)
        acc += bo[:, None]
        out[b, 48:64] = acc.reshape(16, DIM, DIM, DIM)
    return out
